# revision 10
# baseline (speedup 1.0000x reference)
"""Trainium2 Bass kernel for nn_BinaryFullTensorCell (gnn_message_passing).

Computes, for each node n:
    out[n,k] = sum_{i,j} h1[n,i]*h2[n,j]*A[i,j,k] + h1@U1_w.T + h2@U2_w.T + U2_b

Sharding: data-parallel over the node axis across 8 NeuronCores (2048
nodes/core); A / U1_w / U2_w / U2_b replicated.

Algorithm (v5):
  - A is mean-centered (A' = A - 0.5); the 0.5*s1[n]*s2[n] restoration
    rides a f32r matmul against a constant 0.5 matrix with rhs
    gt[j,n] = h2[n,j]*s1[n] computed on host.
  - i-rows 0..87 use fp8e4: o8[j,c,n] = e4m3(bf16(h1[c,n]*h2[j,n]))
    contracted against e4m3(A') with DoubleRow pairs (2 rows/instr);
    rows 88..127 stay bf16 end-to-end.  Error budget: rel_fro ~0.0191
    (gate 2e-2); measured on HW every 512-col matmul costs 512 PE
    cycles regardless of mode, so DR pairs are the only PE lever.
  - h1 rows are partition-broadcast by stride-0 DMA reads of a packed
    [nb, cb, 16, 512] bf16 layout; the o-mult runs on DVE at 2x into
    bf16, fp8 rows are then quantized on ACT.
  - A (fp8 + bf16 parts) is SBUF-resident, loaded once per chunk tile.
  - Linear terms and the mean matmul accumulate into the same PSUM
    group; bias added on ACT eviction; output stored transposed
    [3H, NS] in bf16, host un-transposes and upcasts.
"""

import numpy as np

N_FULL = 16384
N_CORES = 8
NS = N_FULL // N_CORES  # 2048 nodes per core
H = 128
KO = 3 * H  # 384
P = 128
HN = 512  # nodes per n-block
NB = NS // HN  # 4 n-blocks
CCH = 16  # c-rows per chunk
NCH = H // CCH  # 8 chunks
QF = 88  # fp8 i-rows total; rest bf16
# per chunk: (n_pairs, n_bf16_rows); alternate 6/5 pairs -> 44 pairs, 40 bf16
CHUNKS = [(6, 4) if _cb % 2 == 0 else (5, 6) for _cb in range(NCH)]
NPAIR = sum(p for p, _ in CHUNKS)  # 44
PAIR_BASE = [sum(p for p, _ in CHUNKS[:_cb]) for _cb in range(NCH)]
B16_BASE = [sum(b for _, b in CHUNKS[:_cb]) for _cb in range(NCH)]
# quantize split per chunk: (n_act, n_gps, n_dve) rows, sum = 2*pairs
QSPLIT = {6: (7, 3, 2), 5: (6, 2, 2)}
WARMUP = 0

_CACHE = {}


def _build_nc():
    import concourse.bacc as bacc
    import concourse.mybir as mybir
    import concourse.tile as tile

    f32 = mybir.dt.float32
    f32r = mybir.dt.float32r
    bf16 = mybir.dt.bfloat16
    f8 = mybir.dt.float8e4
    DR = mybir.MatmulPerfMode.DoubleRow
    AF = mybir.ActivationFunctionType
    nc = bacc.Bacc("TRN2", target_bir_lowering=False, debug=False)

    a8d = nc.dram_tensor("a8d", [P, NPAIR, 2, KO], f8, kind="ExternalInput")
    a16d = nc.dram_tensor("a16d", [P, H - QF, KO], bf16, kind="ExternalInput")
    # h1bc[nb, cb, c, n] = h1T[cb*16+c, nb*512+n] (bf16, contiguous chunks)
    h1bc = nc.dram_tensor("h1bc", [NB, NCH, CCH, HN], bf16, kind="ExternalInput")
    h2tb = nc.dram_tensor("h2tb", [H, NS], bf16, kind="ExternalInput")
    h1tb = nc.dram_tensor("h1tb", [H, NS], bf16, kind="ExternalInput")
    gtd = nc.dram_tensor("gtd", [P, NS], bf16, kind="ExternalInput")
    u1t = nc.dram_tensor("u1t", [H, 3, P], bf16, kind="ExternalInput")
    u2t = nc.dram_tensor("u2t", [H, 3, P], bf16, kind="ExternalInput")
    biasw = nc.dram_tensor("biasw", [P, 3], f32, kind="ExternalInput")
    outT = nc.dram_tensor("outT", [KO, NS], bf16, kind="ExternalOutput")

    with tile.TileContext(nc) as tc:
        with tc.tile_pool(name="consts", bufs=1) as consts:
            h2tb_sb = consts.tile([P, NS], bf16)
            for q in range(2):
                nc.sync.dma_start(
                    out=h2tb_sb[:, q * 1024 : (q + 1) * 1024],
                    in_=h2tb.ap()[:, q * 1024 : (q + 1) * 1024],
                )
            # A tiles per chunk so early matmuls only wait on their slice
            a8_sb, a16_sb = [], []
            for cb, (nprs, nb16) in enumerate(CHUNKS):
                t8 = t16 = None
                if nprs:
                    t8 = consts.tile(
                        [P, nprs, 2, KO], f8, tag=f"a8c{cb}", name=f"a8c{cb}"
                    )
                    p0 = PAIR_BASE[cb]
                    h = (nprs + 1) // 2
                    for q0, q1 in ((0, h), (h, nprs)):
                        nc.sync.dma_start(
                            out=t8[:, q0:q1],
                            in_=a8d.ap()[:, p0 + q0 : p0 + q1],
                        )
                if nb16:
                    t16 = consts.tile(
                        [P, nb16, KO], bf16, tag=f"a16c{cb}", name=f"a16c{cb}"
                    )
                    t0 = B16_BASE[cb]
                    h = (nb16 + 1) // 2
                    for q0, q1 in ((0, h), (h, nb16)):
                        nc.sync.dma_start(
                            out=t16[:, q0:q1],
                            in_=a16d.ap()[:, t0 + q0 : t0 + q1],
                        )
                a8_sb.append(t8)
                a16_sb.append(t16)

            h1tb_sb = consts.tile([P, NS], bf16)
            for q in range(2):
                nc.sync.dma_start(
                    out=h1tb_sb[:, q * 1024 : (q + 1) * 1024],
                    in_=h1tb.ap()[:, q * 1024 : (q + 1) * 1024],
                )
            gt_sb = consts.tile([P, NS], bf16)
            for q in range(2):
                nc.sync.dma_start(
                    out=gt_sb[:, q * 1024 : (q + 1) * 1024],
                    in_=gtd.ap()[:, q * 1024 : (q + 1) * 1024],
                )
            u1t_sb = consts.tile([P, 3, P], bf16)
            nc.sync.dma_start(out=u1t_sb[:], in_=u1t.ap())
            u2t_sb = consts.tile([P, 3, P], bf16)
            nc.sync.dma_start(out=u2t_sb[:], in_=u2t.ap())
            bias_sb = consts.tile([P, 3], f32)
            nc.sync.dma_start(out=bias_sb[:], in_=biasw.ap())

            half_sb = consts.tile([P, P], bf16)
            nc.vector.memset(half_sb[:], 0.5)

            with (
                tc.tile_pool(name="bc_pool", bufs=3) as bc_pool,
                tc.tile_pool(name="prod_pool", bufs=3) as prod_pool,
                tc.tile_pool(name="o8_pool", bufs=3) as o8_pool,
                tc.tile_pool(name="acc_ps", bufs=6, space="PSUM") as acc_ps,
                tc.tile_pool(name="warm_ps", bufs=1, space="PSUM") as warm_ps,
                tc.tile_pool(name="osb_pool", bufs=2) as osb_pool,
            ):
                # PE warm-up: push DVFS to the high p-state while the
                # first DMAs land.  Results discarded.
                if WARMUP:
                    wsrc = consts.tile([P, 2, 256], f8)
                    nc.vector.memset(wsrc[:], 0.25)
                    wps = warm_ps.tile([P, 512], f32, tag="warm", name="warm")
                    for w in range(WARMUP):
                        nc.tensor.matmul(
                            wps[:, :256],
                            wsrc[:, :, :P],
                            wsrc[:],
                            start=True,
                            stop=True,
                            perf_mode=DR,
                        )

                for nb in range(NB):
                    n0 = nb * HN
                    acc = [
                        acc_ps.tile([P, HN], f32, tag="acc", name=f"acc{nb}_{m}")
                        for m in range(3)
                    ]
                    for cb, (nprs, nb16) in enumerate(CHUNKS):
                        bc = bc_pool.tile([P, CCH, HN], bf16, tag="bc")
                        for q in range(4):
                            nc.sync.dma_start(
                                out=bc[:, q * 4 : (q + 1) * 4, :],
                                in_=h1bc.ap()[nb, cb][q * 4 : (q + 1) * 4]
                                .unsqueeze(0)
                                .broadcast_to([P, 4, HN]),
                            )
                        prod = prod_pool.tile([P, CCH, HN], bf16, tag="prod")
                        if nprs:
                            nf = 2 * nprs
                            qa, qg, qd = QSPLIT[nprs]
                            o8 = o8_pool.tile([P, CCH, HN], f8, tag="o8")
                            # rows [0, nf-qd) + bf16 rows via bf16 prod
                            nc.vector.tensor_mul(
                                prod[:, : nf - qd, :],
                                bc[:, : nf - qd, :],
                                h2tb_sb[:, None, n0 : n0 + HN].broadcast_to(
                                    [P, nf - qd, HN]
                                ),
                            )
                            if nb16:
                                nc.vector.tensor_mul(
                                    prod[:, nf:, :],
                                    bc[:, nf:, :],
                                    h2tb_sb[:, None, n0 : n0 + HN].broadcast_to(
                                        [P, nb16, HN]
                                    ),
                                )
                            # rows [nf-qd, nf): DVE direct multiply to fp8
                            nc.vector.tensor_mul(
                                o8[:, nf - qd : nf, :],
                                bc[:, nf - qd : nf, :],
                                h2tb_sb[:, None, n0 : n0 + HN].broadcast_to(
                                    [P, qd, HN]
                                ),
                            )
                            nc.scalar.copy(o8[:, :qa, :], prod[:, :qa, :])
                            nc.gpsimd.tensor_copy(
                                o8[:, qa : qa + qg, :], prod[:, qa : qa + qg, :]
                            )
                            for pp in range(nprs):
                                for m in range(3):
                                    nc.tensor.matmul(
                                        acc[m][:],
                                        a8_sb[cb][:, pp, :, m * P : (m + 1) * P],
                                        o8[:, 2 * pp : 2 * pp + 2, :],
                                        start=(cb == 0 and pp == 0),
                                        stop=False,
                                        perf_mode=DR,
                                    )
                        for t in range(nb16):
                            tl = t + 2 * nprs  # bf16 rows follow fp8 rows
                            for m in range(3):
                                nc.tensor.matmul(
                                    acc[m][:],
                                    a16_sb[cb][:, t, m * P : (m + 1) * P],
                                    prod[:, tl, :],
                                    start=False,
                                    stop=False,
                                )
                    for m in range(3):
                        nc.tensor.matmul(
                            acc[m][:],
                            u1t_sb[:, m, :],
                            h1tb_sb[:, n0 : n0 + HN],
                            start=False,
                            stop=False,
                        )
                        nc.tensor.matmul(
                            acc[m][:],
                            u2t_sb[:, m, :],
                            h2tb_sb[:, n0 : n0 + HN],
                            start=False,
                            stop=False,
                        )
                        nc.tensor.matmul(
                            acc[m][:],
                            half_sb[:],
                            gt_sb[:, n0 : n0 + HN],
                            start=False,
                            stop=True,
                        )
                        osb = osb_pool.tile([P, HN], bf16, tag="osb")
                        nc.scalar.activation(
                            osb[:], acc[m][:], AF.Identity, bias=bias_sb[:, m : m + 1]
                        )
                        nc.sync.dma_start(
                            out=outT.ap()[m * P : (m + 1) * P, n0 : n0 + HN],
                            in_=osb[:],
                        )

    nc.compile()
    return nc


def _get_nc():
    if "nc" not in _CACHE:
        _CACHE["nc"] = _build_nc()
    return _CACHE["nc"]


def _prep_full(inputs):
    import ml_dtypes

    f8 = ml_dtypes.float8_e4m3
    bf = ml_dtypes.bfloat16
    nhf = np.ascontiguousarray(np.asarray(inputs["neighbour_h"], dtype=np.float32))
    A = np.asarray(inputs["A"], dtype=np.float32)
    U1 = np.asarray(inputs["U1_w"], dtype=np.float32)
    U2 = np.asarray(inputs["U2_w"], dtype=np.float32)
    U2b = np.asarray(inputs["U2_b"], dtype=np.float32)

    Ac = (A - 0.5).astype(np.float32)
    # chunk cb: natural rows [cb*16, cb*16+2p) are fp8 DR pairs, rest bf16
    f8_rows = np.array(
        [cb * CCH + r for cb, (p, _) in enumerate(CHUNKS) for r in range(2 * p)]
    )
    b16_rows = np.array(
        [cb * CCH + r for cb, (p, b) in enumerate(CHUNKS) for r in range(2 * p, CCH)]
    )
    a8 = np.ascontiguousarray(
        Ac[f8_rows].reshape(NPAIR, 2, H, KO).transpose(2, 0, 1, 3)
    ).astype(f8)
    a16 = np.ascontiguousarray(Ac[b16_rows].transpose(1, 0, 2)).astype(bf)

    h1 = nhf[:, 0, :]
    h2 = nhf[:, 1, :]
    h1T = np.ascontiguousarray(h1.T).astype(bf)  # [H, N]
    h2T = np.ascontiguousarray(h2.T).astype(bf)
    # h1bc[core][nb, cb, c, n]
    h1bc = np.ascontiguousarray(
        h1T.reshape(NCH, CCH, N_CORES, NB, HN).transpose(2, 3, 0, 1, 4)
    )
    # gt[core][j, n] = h2[n, j] * s1[n], f32
    s1 = h1.astype(np.float64).sum(axis=1).astype(np.float32)
    gt = np.ascontiguousarray((h2 * s1[:, None]).T).astype(bf)  # [H, N]

    u1t = np.ascontiguousarray(U1.reshape(3, P, H).transpose(2, 0, 1)).astype(bf)
    u2t = np.ascontiguousarray(U2.reshape(3, P, H).transpose(2, 0, 1)).astype(bf)
    biasw = np.ascontiguousarray(U2b.reshape(3, P).T)

    return a8, a16, h1bc, h1T, h2T, gt, u1t, u2t, biasw


def make_in_maps(inputs):
    a8, a16, h1bc, h1T, h2T, gt, u1t, u2t, biasw = _prep_full(inputs)
    return [
        {
            "a8d": a8,
            "a16d": a16,
            "h1bc": h1bc[i],
            "h2tb": np.ascontiguousarray(h2T[:, i * NS : (i + 1) * NS]),
            "h1tb": np.ascontiguousarray(h1T[:, i * NS : (i + 1) * NS]),
            "gtd": np.ascontiguousarray(gt[:, i * NS : (i + 1) * NS]),
            "u1t": u1t,
            "u2t": u2t,
            "biasw": biasw,
        }
        for i in range(N_CORES)
    ]


def kernel(**inputs: np.ndarray) -> np.ndarray:
    in_maps = make_in_maps(inputs)
    nc = _get_nc()
    from concourse import bass2jax

    results = bass2jax.run_bass_via_pjrt(nc, in_maps, n_cores=N_CORES)
    return np.concatenate(
        [np.asarray(results[i]["outT"]).astype(np.float32).T for i in range(N_CORES)],
        axis=0,
    )


if __name__ == "__main__":
    rng = np.random.default_rng(0)
    ins = {
        "neighbour_h": rng.standard_normal((N_FULL, 2, H), dtype=np.float32),
        "A": rng.random((H, H, KO), dtype=np.float32),
        "U1_w": rng.standard_normal((KO, H), dtype=np.float32),
        "U2_w": rng.standard_normal((KO, H), dtype=np.float32),
        "U2_b": rng.standard_normal((KO,), dtype=np.float32),
    }
    out = kernel(**ins)
    h1 = ins["neighbour_h"][:, 0, :].astype(np.float64)
    h2 = ins["neighbour_h"][:, 1, :].astype(np.float64)
    A = ins["A"].astype(np.float64)
    outer = np.einsum("ni,nj->nij", h1, h2).reshape(N_FULL, H * H)
    exp = (
        outer @ A.reshape(H * H, KO)
        + h1 @ ins["U1_w"].T.astype(np.float64)
        + h2 @ ins["U2_w"].T.astype(np.float64)
        + ins["U2_b"].astype(np.float64)
    )
    err = np.linalg.norm(out - exp) / np.linalg.norm(exp)
    print("kernel output", out.shape, out.dtype, "rel fro err:", err)


# revision 11
# speedup vs baseline: 1.2122x; 1.2122x over previous
"""Trainium2 Bass kernel for nn_BinaryFullTensorCell (gnn_message_passing).

Computes, for each node n:
    out[n,k] = sum_{i,j} h1[n,i]*h2[n,j]*A[i,j,k] + h1@U1_w.T + h2@U2_w.T + U2_b

Sharding: data-parallel over the node axis across 8 NeuronCores (2048
nodes/core); A / U1_w / U2_w / U2_b replicated.

Algorithm (v5):
  - A is mean-centered (A' = A - 0.5); the 0.5*s1[n]*s2[n] restoration
    rides a f32r matmul against a constant 0.5 matrix with rhs
    gt[j,n] = h2[n,j]*s1[n] computed on host.
  - i-rows 0..87 use fp8e4: o8[j,c,n] = e4m3(bf16(h1[c,n]*h2[j,n]))
    contracted against e4m3(A') with DoubleRow pairs (2 rows/instr);
    rows 88..127 stay bf16 end-to-end.  Error budget: rel_fro ~0.0191
    (gate 2e-2); measured on HW every 512-col matmul costs 512 PE
    cycles regardless of mode, so DR pairs are the only PE lever.
  - h1 rows are partition-broadcast by stride-0 DMA reads of a packed
    [nb, cb, 16, 512] bf16 layout; the o-mult runs on DVE at 2x into
    bf16, fp8 rows are then quantized on ACT.
  - A (fp8 + bf16 parts) is SBUF-resident, loaded once per chunk tile.
  - Linear terms and the mean matmul accumulate into the same PSUM
    group; bias added on ACT eviction; output stored transposed
    [3H, NS] in bf16, host un-transposes and upcasts.
"""

import numpy as np

N_FULL = 16384
N_CORES = 8
NS = N_FULL // N_CORES  # 2048 nodes per core
H = 128
KO = 3 * H  # 384
P = 128
HN = 512  # nodes per n-block
NB = NS // HN  # 4 n-blocks
CCH = 16  # c-rows per chunk
NCH = H // CCH  # 8 chunks
QF = 88  # fp8 i-rows total; rest bf16
# per chunk: (n_pairs, n_bf16_rows); alternate 6/5 pairs -> 44 pairs, 40 bf16
CHUNKS = [(6, 4) if _cb % 2 == 0 else (5, 6) for _cb in range(NCH)]
NPAIR = sum(p for p, _ in CHUNKS)  # 44
PAIR_BASE = [sum(p for p, _ in CHUNKS[:_cb]) for _cb in range(NCH)]
B16_BASE = [sum(b for _, b in CHUNKS[:_cb]) for _cb in range(NCH)]
WARMUP = 0

_CACHE = {}


def _build_nc():
    import concourse.bacc as bacc
    import concourse.mybir as mybir
    import concourse.tile as tile

    f32 = mybir.dt.float32
    f32r = mybir.dt.float32r
    bf16 = mybir.dt.bfloat16
    f8 = mybir.dt.float8e4
    DR = mybir.MatmulPerfMode.DoubleRow
    AF = mybir.ActivationFunctionType
    nc = bacc.Bacc("TRN2", target_bir_lowering=False, debug=False)

    a8d = nc.dram_tensor("a8d", [P, NPAIR, 2, KO], f8, kind="ExternalInput")
    a16d = nc.dram_tensor("a16d", [P, H - QF, KO], bf16, kind="ExternalInput")
    # h1bc[nb, cb, c, n] = h1T[cb*16+c, nb*512+n] (bf16, contiguous chunks)
    h1bc = nc.dram_tensor("h1bc", [NB, NCH, CCH, HN], bf16, kind="ExternalInput")
    h2tb = nc.dram_tensor("h2tb", [H, NS], bf16, kind="ExternalInput")
    h1tb = nc.dram_tensor("h1tb", [H, NS], bf16, kind="ExternalInput")
    gtd = nc.dram_tensor("gtd", [P, NS], bf16, kind="ExternalInput")
    u1t = nc.dram_tensor("u1t", [H, 3, P], bf16, kind="ExternalInput")
    u2t = nc.dram_tensor("u2t", [H, 3, P], bf16, kind="ExternalInput")
    biasw = nc.dram_tensor("biasw", [P, 3], f32, kind="ExternalInput")
    outT = nc.dram_tensor("outT", [KO, NS], bf16, kind="ExternalOutput")

    with tile.TileContext(nc) as tc:
        with tc.tile_pool(name="consts", bufs=1) as consts:
            h2tb_sb = consts.tile([P, NS], bf16)
            for q in range(2):
                nc.sync.dma_start(
                    out=h2tb_sb[:, q * 1024 : (q + 1) * 1024],
                    in_=h2tb.ap()[:, q * 1024 : (q + 1) * 1024],
                )
            # A tiles per chunk so early matmuls only wait on their slice
            a8_sb, a16_sb = [], []
            for cb, (nprs, nb16) in enumerate(CHUNKS):
                t8 = t16 = None
                if nprs:
                    t8 = consts.tile(
                        [P, nprs, 2, KO], f8, tag=f"a8c{cb}", name=f"a8c{cb}"
                    )
                    p0 = PAIR_BASE[cb]
                    h = (nprs + 1) // 2
                    for q0, q1 in ((0, h), (h, nprs)):
                        nc.sync.dma_start(
                            out=t8[:, q0:q1],
                            in_=a8d.ap()[:, p0 + q0 : p0 + q1],
                        )
                if nb16:
                    t16 = consts.tile(
                        [P, nb16, KO], bf16, tag=f"a16c{cb}", name=f"a16c{cb}"
                    )
                    t0 = B16_BASE[cb]
                    h = (nb16 + 1) // 2
                    for q0, q1 in ((0, h), (h, nb16)):
                        nc.sync.dma_start(
                            out=t16[:, q0:q1],
                            in_=a16d.ap()[:, t0 + q0 : t0 + q1],
                        )
                a8_sb.append(t8)
                a16_sb.append(t16)

            h1tb_sb = consts.tile([P, NS], bf16)
            for q in range(2):
                nc.sync.dma_start(
                    out=h1tb_sb[:, q * 1024 : (q + 1) * 1024],
                    in_=h1tb.ap()[:, q * 1024 : (q + 1) * 1024],
                )
            gt_sb = consts.tile([P, NS], bf16)
            for q in range(2):
                nc.sync.dma_start(
                    out=gt_sb[:, q * 1024 : (q + 1) * 1024],
                    in_=gtd.ap()[:, q * 1024 : (q + 1) * 1024],
                )
            u1t_sb = consts.tile([P, 3, P], bf16)
            nc.sync.dma_start(out=u1t_sb[:], in_=u1t.ap())
            u2t_sb = consts.tile([P, 3, P], bf16)
            nc.sync.dma_start(out=u2t_sb[:], in_=u2t.ap())
            bias_sb = consts.tile([P, 3], f32)
            nc.sync.dma_start(out=bias_sb[:], in_=biasw.ap())

            half_sb = consts.tile([P, P], bf16)
            nc.vector.memset(half_sb[:], 0.5)

            with (
                tc.tile_pool(name="bc_pool", bufs=3) as bc_pool,
                tc.tile_pool(name="prod_pool", bufs=3) as prod_pool,
                tc.tile_pool(name="o8_pool", bufs=3) as o8_pool,
                tc.tile_pool(name="acc_ps", bufs=6, space="PSUM") as acc_ps,
                tc.tile_pool(name="warm_ps", bufs=1, space="PSUM") as warm_ps,
                tc.tile_pool(name="osb_pool", bufs=2) as osb_pool,
            ):
                # PE warm-up: push DVFS to the high p-state while the
                # first DMAs land.  Results discarded.
                if WARMUP:
                    wsrc = consts.tile([P, 2, 256], f8)
                    nc.vector.memset(wsrc[:], 0.25)
                    wps = warm_ps.tile([P, 512], f32, tag="warm", name="warm")
                    for w in range(WARMUP):
                        nc.tensor.matmul(
                            wps[:, :256],
                            wsrc[:, :, :P],
                            wsrc[:],
                            start=True,
                            stop=True,
                            perf_mode=DR,
                        )

                for nb in range(NB):
                    n0 = nb * HN
                    acc = [
                        acc_ps.tile([P, HN], f32, tag="acc", name=f"acc{nb}_{m}")
                        for m in range(3)
                    ]
                    for cb, (nprs, nb16) in enumerate(CHUNKS):
                        nf = 2 * nprs
                        bca = bc_pool.tile([P, nf, HN], bf16, tag="bca")
                        for q0 in range(0, nf, 4):
                            q1 = min(q0 + 4, nf)
                            nc.sync.dma_start(
                                out=bca[:, q0:q1, :],
                                in_=h1bc.ap()[nb, cb][q0:q1]
                                .unsqueeze(0)
                                .broadcast_to([P, q1 - q0, HN]),
                            )
                        bcb = bc_pool.tile([P, nb16, HN], bf16, tag="bcb")
                        for q0 in range(0, nb16, 4):
                            q1 = min(q0 + 4, nb16)
                            nc.sync.dma_start(
                                out=bcb[:, q0:q1, :],
                                in_=h1bc.ap()[nb, cb][nf + q0 : nf + q1]
                                .unsqueeze(0)
                                .broadcast_to([P, q1 - q0, HN]),
                            )
                        proda = prod_pool.tile([P, nf, HN], bf16, tag="proda")
                        nc.vector.tensor_mul(
                            proda[:],
                            bca[:],
                            h2tb_sb[:, None, n0 : n0 + HN].broadcast_to(
                                [P, nf, HN]
                            ),
                        )
                        prodb = prod_pool.tile([P, nb16, HN], bf16, tag="prodb")
                        nc.vector.tensor_mul(
                            prodb[:],
                            bcb[:],
                            h2tb_sb[:, None, n0 : n0 + HN].broadcast_to(
                                [P, nb16, HN]
                            ),
                        )
                        o8 = o8_pool.tile([P, nf, HN], f8, tag="o8")
                        nc.scalar.copy(o8[:], proda[:])
                        for pp in range(nprs):
                            for m in range(3):
                                nc.tensor.matmul(
                                    acc[m][:],
                                    a8_sb[cb][:, pp, :, m * P : (m + 1) * P],
                                    o8[:, 2 * pp : 2 * pp + 2, :],
                                    start=(cb == 0 and pp == 0),
                                    stop=False,
                                    perf_mode=DR,
                                )
                        for t in range(nb16):
                            for m in range(3):
                                nc.tensor.matmul(
                                    acc[m][:],
                                    a16_sb[cb][:, t, m * P : (m + 1) * P],
                                    prodb[:, t, :],
                                    start=False,
                                    stop=False,
                                )
                    for m in range(3):
                        nc.tensor.matmul(
                            acc[m][:],
                            u1t_sb[:, m, :],
                            h1tb_sb[:, n0 : n0 + HN],
                            start=False,
                            stop=False,
                        )
                        nc.tensor.matmul(
                            acc[m][:],
                            u2t_sb[:, m, :],
                            h2tb_sb[:, n0 : n0 + HN],
                            start=False,
                            stop=False,
                        )
                        nc.tensor.matmul(
                            acc[m][:],
                            half_sb[:],
                            gt_sb[:, n0 : n0 + HN],
                            start=False,
                            stop=True,
                        )
                        osb = osb_pool.tile([P, HN], bf16, tag="osb")
                        nc.scalar.activation(
                            osb[:], acc[m][:], AF.Identity, bias=bias_sb[:, m : m + 1]
                        )
                        nc.sync.dma_start(
                            out=outT.ap()[m * P : (m + 1) * P, n0 : n0 + HN],
                            in_=osb[:],
                        )

    nc.compile()
    return nc


def _get_nc():
    if "nc" not in _CACHE:
        _CACHE["nc"] = _build_nc()
    return _CACHE["nc"]


def _prep_full(inputs):
    import ml_dtypes

    f8 = ml_dtypes.float8_e4m3
    bf = ml_dtypes.bfloat16
    nhf = np.ascontiguousarray(np.asarray(inputs["neighbour_h"], dtype=np.float32))
    A = np.asarray(inputs["A"], dtype=np.float32)
    U1 = np.asarray(inputs["U1_w"], dtype=np.float32)
    U2 = np.asarray(inputs["U2_w"], dtype=np.float32)
    U2b = np.asarray(inputs["U2_b"], dtype=np.float32)

    Ac = (A - 0.5).astype(np.float32)
    # chunk cb: natural rows [cb*16, cb*16+2p) are fp8 DR pairs, rest bf16
    f8_rows = np.array(
        [cb * CCH + r for cb, (p, _) in enumerate(CHUNKS) for r in range(2 * p)]
    )
    b16_rows = np.array(
        [cb * CCH + r for cb, (p, b) in enumerate(CHUNKS) for r in range(2 * p, CCH)]
    )
    a8 = np.ascontiguousarray(
        Ac[f8_rows].reshape(NPAIR, 2, H, KO).transpose(2, 0, 1, 3)
    ).astype(f8)
    a16 = np.ascontiguousarray(Ac[b16_rows].transpose(1, 0, 2)).astype(bf)

    h1 = nhf[:, 0, :]
    h2 = nhf[:, 1, :]
    h1T = np.ascontiguousarray(h1.T).astype(bf)  # [H, N]
    h2T = np.ascontiguousarray(h2.T).astype(bf)
    # h1bc[core][nb, cb, c, n]
    h1bc = np.ascontiguousarray(
        h1T.reshape(NCH, CCH, N_CORES, NB, HN).transpose(2, 3, 0, 1, 4)
    )
    # gt[core][j, n] = h2[n, j] * s1[n], f32
    s1 = h1.astype(np.float64).sum(axis=1).astype(np.float32)
    gt = np.ascontiguousarray((h2 * s1[:, None]).T).astype(bf)  # [H, N]

    u1t = np.ascontiguousarray(U1.reshape(3, P, H).transpose(2, 0, 1)).astype(bf)
    u2t = np.ascontiguousarray(U2.reshape(3, P, H).transpose(2, 0, 1)).astype(bf)
    biasw = np.ascontiguousarray(U2b.reshape(3, P).T)

    return a8, a16, h1bc, h1T, h2T, gt, u1t, u2t, biasw


def make_in_maps(inputs):
    a8, a16, h1bc, h1T, h2T, gt, u1t, u2t, biasw = _prep_full(inputs)
    return [
        {
            "a8d": a8,
            "a16d": a16,
            "h1bc": h1bc[i],
            "h2tb": np.ascontiguousarray(h2T[:, i * NS : (i + 1) * NS]),
            "h1tb": np.ascontiguousarray(h1T[:, i * NS : (i + 1) * NS]),
            "gtd": np.ascontiguousarray(gt[:, i * NS : (i + 1) * NS]),
            "u1t": u1t,
            "u2t": u2t,
            "biasw": biasw,
        }
        for i in range(N_CORES)
    ]


def kernel(**inputs: np.ndarray) -> np.ndarray:
    in_maps = make_in_maps(inputs)
    nc = _get_nc()
    from concourse import bass2jax

    results = bass2jax.run_bass_via_pjrt(nc, in_maps, n_cores=N_CORES)
    return np.concatenate(
        [np.asarray(results[i]["outT"]).astype(np.float32).T for i in range(N_CORES)],
        axis=0,
    )


if __name__ == "__main__":
    rng = np.random.default_rng(0)
    ins = {
        "neighbour_h": rng.standard_normal((N_FULL, 2, H), dtype=np.float32),
        "A": rng.random((H, H, KO), dtype=np.float32),
        "U1_w": rng.standard_normal((KO, H), dtype=np.float32),
        "U2_w": rng.standard_normal((KO, H), dtype=np.float32),
        "U2_b": rng.standard_normal((KO,), dtype=np.float32),
    }
    out = kernel(**ins)
    h1 = ins["neighbour_h"][:, 0, :].astype(np.float64)
    h2 = ins["neighbour_h"][:, 1, :].astype(np.float64)
    A = ins["A"].astype(np.float64)
    outer = np.einsum("ni,nj->nij", h1, h2).reshape(N_FULL, H * H)
    exp = (
        outer @ A.reshape(H * H, KO)
        + h1 @ ins["U1_w"].T.astype(np.float64)
        + h2 @ ins["U2_w"].T.astype(np.float64)
        + ins["U2_b"].astype(np.float64)
    )
    err = np.linalg.norm(out - exp) / np.linalg.norm(exp)
    print("kernel output", out.shape, out.dtype, "rel fro err:", err)


# revision 13
# speedup vs baseline: 1.2284x; 1.0133x over previous
"""Trainium2 Bass kernel for nn_BinaryFullTensorCell (gnn_message_passing).

Computes, for each node n:
    out[n,k] = sum_{i,j} h1[n,i]*h2[n,j]*A[i,j,k] + h1@U1_w.T + h2@U2_w.T + U2_b

Sharding: data-parallel over the node axis across 8 NeuronCores (2048
nodes/core); A / U1_w / U2_w / U2_b replicated.

Algorithm (v5):
  - A is mean-centered (A' = A - 0.5); the 0.5*s1[n]*s2[n] restoration
    rides a f32r matmul against a constant 0.5 matrix with rhs
    gt[j,n] = h2[n,j]*s1[n] computed on host.
  - i-rows 0..87 use fp8e4: o8[j,c,n] = e4m3(bf16(h1[c,n]*h2[j,n]))
    contracted against e4m3(A') with DoubleRow pairs (2 rows/instr);
    rows 88..127 stay bf16 end-to-end.  Error budget: rel_fro ~0.0191
    (gate 2e-2); measured on HW every 512-col matmul costs 512 PE
    cycles regardless of mode, so DR pairs are the only PE lever.
  - h1 rows are partition-broadcast by stride-0 DMA reads of a packed
    [nb, cb, 16, 512] bf16 layout; the o-mult runs on DVE at 2x into
    bf16, fp8 rows are then quantized on ACT.
  - A (fp8 + bf16 parts) is SBUF-resident, loaded once per chunk tile.
  - Linear terms and the mean matmul accumulate into the same PSUM
    group; bias added on ACT eviction; output stored transposed
    [3H, NS] in bf16, host un-transposes and upcasts.
"""

import numpy as np

N_FULL = 16384
N_CORES = 8
NS = N_FULL // N_CORES  # 2048 nodes per core
H = 128
KO = 3 * H  # 384
P = 128
HN = 512  # nodes per n-block
NB = NS // HN  # 4 n-blocks
CCH = 16  # c-rows per chunk
NCH = H // CCH  # 8 chunks
QF = 80  # fp8 i-rows total; rest bf16
# per chunk: (n_pairs, n_bf16_rows) -- uniform so pool tiles are one size
CHUNKS = [(5, 6) for _cb in range(NCH)]
NPAIR = sum(p for p, _ in CHUNKS)  # 44
PAIR_BASE = [sum(p for p, _ in CHUNKS[:_cb]) for _cb in range(NCH)]
B16_BASE = [sum(b for _, b in CHUNKS[:_cb]) for _cb in range(NCH)]
WARMUP = 16

_CACHE = {}


def _build_nc():
    import concourse.bacc as bacc
    import concourse.mybir as mybir
    import concourse.tile as tile

    f32 = mybir.dt.float32
    f32r = mybir.dt.float32r
    bf16 = mybir.dt.bfloat16
    f8 = mybir.dt.float8e4
    DR = mybir.MatmulPerfMode.DoubleRow
    AF = mybir.ActivationFunctionType
    nc = bacc.Bacc("TRN2", target_bir_lowering=False, debug=False)

    a8d = nc.dram_tensor("a8d", [P, NPAIR, 2, KO], f8, kind="ExternalInput")
    a16d = nc.dram_tensor("a16d", [P, H - QF, KO], bf16, kind="ExternalInput")
    # h1bc[nb, cb, c, n] = h1T[cb*16+c, nb*512+n] (bf16, contiguous chunks)
    h1bc = nc.dram_tensor("h1bc", [NB, NCH, CCH, HN], bf16, kind="ExternalInput")
    h2tb = nc.dram_tensor("h2tb", [H, NS], bf16, kind="ExternalInput")
    h1tb = nc.dram_tensor("h1tb", [H, NS], bf16, kind="ExternalInput")
    gtd = nc.dram_tensor("gtd", [P, NS], bf16, kind="ExternalInput")
    u1t = nc.dram_tensor("u1t", [H, 3, P], bf16, kind="ExternalInput")
    u2t = nc.dram_tensor("u2t", [H, 3, P], bf16, kind="ExternalInput")
    biasw = nc.dram_tensor("biasw", [P, 3], f32, kind="ExternalInput")
    outT = nc.dram_tensor("outT", [KO, NS], bf16, kind="ExternalOutput")

    with tile.TileContext(nc) as tc:
        with tc.tile_pool(name="consts", bufs=1) as consts:
            h2tb_sb = consts.tile([P, NS], bf16)
            nc.sync.dma_start(
                out=h2tb_sb[:, :HN], in_=h2tb.ap()[:, :HN]
            )

            a8_sb = [None] * NCH
            a16_sb = [None] * NCH

            def _load_a8(cb):
                nprs = CHUNKS[cb][0]
                t8 = consts.tile(
                    [P, nprs, 2, KO], f8, tag=f"a8c{cb}", name=f"a8c{cb}"
                )
                p0 = PAIR_BASE[cb]
                h = (nprs + 1) // 2
                for q0, q1 in ((0, h), (h, nprs)):
                    nc.sync.dma_start(
                        out=t8[:, q0:q1],
                        in_=a8d.ap()[:, p0 + q0 : p0 + q1],
                    )
                a8_sb[cb] = t8

            def _load_a16(cb):
                nb16 = CHUNKS[cb][1]
                t16 = consts.tile(
                    [P, nb16, KO], bf16, tag=f"a16c{cb}", name=f"a16c{cb}"
                )
                t0 = B16_BASE[cb]
                h = (nb16 + 1) // 2
                for q0, q1 in ((0, h), (h, nb16)):
                    nc.sync.dma_start(
                        out=t16[:, q0:q1],
                        in_=a16d.ap()[:, t0 + q0 : t0 + q1],
                    )
                a16_sb[cb] = t16

            _load_a8(0)

            h1tb_sb = consts.tile([P, NS], bf16)
            gt_sb = consts.tile([P, NS], bf16)
            u1t_sb = consts.tile([P, 3, P], bf16)
            u2t_sb = consts.tile([P, 3, P], bf16)
            bias_sb = consts.tile([P, 3], f32)

            def _load_tail_consts():
                for q in range(3):
                    nc.sync.dma_start(
                        out=h2tb_sb[:, (q + 1) * HN : (q + 2) * HN],
                        in_=h2tb.ap()[:, (q + 1) * HN : (q + 2) * HN],
                    )
                for q in range(2):
                    nc.sync.dma_start(
                        out=h1tb_sb[:, q * 1024 : (q + 1) * 1024],
                        in_=h1tb.ap()[:, q * 1024 : (q + 1) * 1024],
                    )
                    nc.sync.dma_start(
                        out=gt_sb[:, q * 1024 : (q + 1) * 1024],
                        in_=gtd.ap()[:, q * 1024 : (q + 1) * 1024],
                    )
                nc.sync.dma_start(out=u1t_sb[:], in_=u1t.ap())
                nc.sync.dma_start(out=u2t_sb[:], in_=u2t.ap())
                nc.sync.dma_start(out=bias_sb[:], in_=biasw.ap())

            half_sb = consts.tile([P, P], bf16)
            nc.vector.memset(half_sb[:], 0.5)

            with (
                tc.tile_pool(name="bc_pool", bufs=3) as bc_pool,
                tc.tile_pool(name="prod_pool", bufs=3) as prod_pool,
                tc.tile_pool(name="o8_pool", bufs=3) as o8_pool,
                tc.tile_pool(name="acc_ps", bufs=6, space="PSUM") as acc_ps,
                tc.tile_pool(name="warm_ps", bufs=1, space="PSUM") as warm_ps,
                tc.tile_pool(name="osb_pool", bufs=2) as osb_pool,
            ):
                # PE warm-up: push DVFS to the high p-state while the
                # first DMAs land.  Results discarded.
                if WARMUP:
                    wsrc = consts.tile([P, 2, 256], f8)
                    nc.vector.memset(wsrc[:], 0.25)
                    wps = warm_ps.tile([P, 512], f32, tag="warm", name="warm")
                    for w in range(WARMUP):
                        nc.tensor.matmul(
                            wps[:, :256],
                            wsrc[:, :, :P],
                            wsrc[:],
                            start=True,
                            stop=True,
                            perf_mode=DR,
                        )

                for nb in range(NB):
                    n0 = nb * HN
                    acc = [
                        acc_ps.tile([P, HN], f32, tag="acc", name=f"acc{nb}_{m}")
                        for m in range(3)
                    ]
                    for cb, (nprs, nb16) in enumerate(CHUNKS):
                        if nb == 0:
                            if a16_sb[cb] is None:
                                _load_a16(cb)
                            if cb + 1 < NCH and a8_sb[cb + 1] is None:
                                _load_a8(cb + 1)
                            if cb == 1:
                                _load_tail_consts()
                        nf = 2 * nprs
                        bca = bc_pool.tile([P, nf, HN], bf16, tag="bca")
                        for q0 in range(0, nf, 4):
                            q1 = min(q0 + 4, nf)
                            nc.sync.dma_start(
                                out=bca[:, q0:q1, :],
                                in_=h1bc.ap()[nb, cb][q0:q1]
                                .unsqueeze(0)
                                .broadcast_to([P, q1 - q0, HN]),
                            )
                        bcb = bc_pool.tile([P, nb16, HN], bf16, tag="bcb")
                        for q0 in range(0, nb16, 4):
                            q1 = min(q0 + 4, nb16)
                            nc.sync.dma_start(
                                out=bcb[:, q0:q1, :],
                                in_=h1bc.ap()[nb, cb][nf + q0 : nf + q1]
                                .unsqueeze(0)
                                .broadcast_to([P, q1 - q0, HN]),
                            )
                        proda = prod_pool.tile([P, nf, HN], bf16, tag="proda")
                        nc.vector.tensor_mul(
                            proda[:],
                            bca[:],
                            h2tb_sb[:, None, n0 : n0 + HN].broadcast_to(
                                [P, nf, HN]
                            ),
                        )
                        prodb = prod_pool.tile([P, nb16, HN], bf16, tag="prodb")
                        nc.vector.tensor_mul(
                            prodb[:],
                            bcb[:],
                            h2tb_sb[:, None, n0 : n0 + HN].broadcast_to(
                                [P, nb16, HN]
                            ),
                        )
                        o8 = o8_pool.tile([P, nf, HN], f8, tag="o8")
                        nc.scalar.copy(o8[:], proda[:])
                        for pp in range(nprs):
                            for m in range(3):
                                nc.tensor.matmul(
                                    acc[m][:],
                                    a8_sb[cb][:, pp, :, m * P : (m + 1) * P],
                                    o8[:, 2 * pp : 2 * pp + 2, :],
                                    start=(cb == 0 and pp == 0),
                                    stop=False,
                                    perf_mode=DR,
                                )
                        for t in range(nb16):
                            for m in range(3):
                                nc.tensor.matmul(
                                    acc[m][:],
                                    a16_sb[cb][:, t, m * P : (m + 1) * P],
                                    prodb[:, t, :],
                                    start=False,
                                    stop=False,
                                )
                    for m in range(3):
                        nc.tensor.matmul(
                            acc[m][:],
                            u1t_sb[:, m, :],
                            h1tb_sb[:, n0 : n0 + HN],
                            start=False,
                            stop=False,
                        )
                        nc.tensor.matmul(
                            acc[m][:],
                            u2t_sb[:, m, :],
                            h2tb_sb[:, n0 : n0 + HN],
                            start=False,
                            stop=False,
                        )
                        nc.tensor.matmul(
                            acc[m][:],
                            half_sb[:],
                            gt_sb[:, n0 : n0 + HN],
                            start=False,
                            stop=True,
                        )
                        osb = osb_pool.tile([P, HN], bf16, tag="osb")
                        nc.scalar.activation(
                            osb[:], acc[m][:], AF.Identity, bias=bias_sb[:, m : m + 1]
                        )
                        nc.sync.dma_start(
                            out=outT.ap()[m * P : (m + 1) * P, n0 : n0 + HN],
                            in_=osb[:],
                        )

    nc.compile()
    return nc


def _get_nc():
    if "nc" not in _CACHE:
        _CACHE["nc"] = _build_nc()
    return _CACHE["nc"]


def _prep_full(inputs):
    import ml_dtypes

    f8 = ml_dtypes.float8_e4m3
    bf = ml_dtypes.bfloat16
    nhf = np.ascontiguousarray(np.asarray(inputs["neighbour_h"], dtype=np.float32))
    A = np.asarray(inputs["A"], dtype=np.float32)
    U1 = np.asarray(inputs["U1_w"], dtype=np.float32)
    U2 = np.asarray(inputs["U2_w"], dtype=np.float32)
    U2b = np.asarray(inputs["U2_b"], dtype=np.float32)

    Ac = (A - 0.5).astype(np.float32)
    # chunk cb: natural rows [cb*16, cb*16+2p) are fp8 DR pairs, rest bf16
    f8_rows = np.array(
        [cb * CCH + r for cb, (p, _) in enumerate(CHUNKS) for r in range(2 * p)]
    )
    b16_rows = np.array(
        [cb * CCH + r for cb, (p, b) in enumerate(CHUNKS) for r in range(2 * p, CCH)]
    )
    a8 = np.ascontiguousarray(
        Ac[f8_rows].reshape(NPAIR, 2, H, KO).transpose(2, 0, 1, 3)
    ).astype(f8)
    a16 = np.ascontiguousarray(Ac[b16_rows].transpose(1, 0, 2)).astype(bf)

    h1 = nhf[:, 0, :]
    h2 = nhf[:, 1, :]
    h1T = np.ascontiguousarray(h1.T).astype(bf)  # [H, N]
    h2T = np.ascontiguousarray(h2.T).astype(bf)
    # h1bc[core][nb, cb, c, n]
    h1bc = np.ascontiguousarray(
        h1T.reshape(NCH, CCH, N_CORES, NB, HN).transpose(2, 3, 0, 1, 4)
    )
    # gt[core][j, n] = h2[n, j] * s1[n], f32
    s1 = h1.astype(np.float64).sum(axis=1).astype(np.float32)
    gt = np.ascontiguousarray((h2 * s1[:, None]).T).astype(bf)  # [H, N]

    u1t = np.ascontiguousarray(U1.reshape(3, P, H).transpose(2, 0, 1)).astype(bf)
    u2t = np.ascontiguousarray(U2.reshape(3, P, H).transpose(2, 0, 1)).astype(bf)
    biasw = np.ascontiguousarray(U2b.reshape(3, P).T)

    return a8, a16, h1bc, h1T, h2T, gt, u1t, u2t, biasw


def make_in_maps(inputs):
    a8, a16, h1bc, h1T, h2T, gt, u1t, u2t, biasw = _prep_full(inputs)
    return [
        {
            "a8d": a8,
            "a16d": a16,
            "h1bc": h1bc[i],
            "h2tb": np.ascontiguousarray(h2T[:, i * NS : (i + 1) * NS]),
            "h1tb": np.ascontiguousarray(h1T[:, i * NS : (i + 1) * NS]),
            "gtd": np.ascontiguousarray(gt[:, i * NS : (i + 1) * NS]),
            "u1t": u1t,
            "u2t": u2t,
            "biasw": biasw,
        }
        for i in range(N_CORES)
    ]


def kernel(**inputs: np.ndarray) -> np.ndarray:
    in_maps = make_in_maps(inputs)
    nc = _get_nc()
    from concourse import bass2jax

    results = bass2jax.run_bass_via_pjrt(nc, in_maps, n_cores=N_CORES)
    return np.concatenate(
        [np.asarray(results[i]["outT"]).astype(np.float32).T for i in range(N_CORES)],
        axis=0,
    )


if __name__ == "__main__":
    rng = np.random.default_rng(0)
    ins = {
        "neighbour_h": rng.standard_normal((N_FULL, 2, H), dtype=np.float32),
        "A": rng.random((H, H, KO), dtype=np.float32),
        "U1_w": rng.standard_normal((KO, H), dtype=np.float32),
        "U2_w": rng.standard_normal((KO, H), dtype=np.float32),
        "U2_b": rng.standard_normal((KO,), dtype=np.float32),
    }
    out = kernel(**ins)
    h1 = ins["neighbour_h"][:, 0, :].astype(np.float64)
    h2 = ins["neighbour_h"][:, 1, :].astype(np.float64)
    A = ins["A"].astype(np.float64)
    outer = np.einsum("ni,nj->nij", h1, h2).reshape(N_FULL, H * H)
    exp = (
        outer @ A.reshape(H * H, KO)
        + h1 @ ins["U1_w"].T.astype(np.float64)
        + h2 @ ins["U2_w"].T.astype(np.float64)
        + ins["U2_b"].astype(np.float64)
    )
    err = np.linalg.norm(out - exp) / np.linalg.norm(exp)
    print("kernel output", out.shape, out.dtype, "rel fro err:", err)
